# revision 31
# baseline (speedup 1.0000x reference)
"""Deformable conv (3x3, pad=1, B=8, Cin=Cout=256, H=W=64) on 8 TRN2 NeuronCores.

Strategy (data-parallel over batch, one image per core):
  1. Prologue per core: build channels-last bf16 copy of x (x_cl[4096+2, 256]) in
     HBM via PE transpose; transpose the (k,c)-ordered weight matrix into lhsT
     layout; compute per-token bilinear indices + corner weights on DVE from the
     offsets.
  2. Main loop over (token-chunk, tap): SWDGE dma_gather fetches, for every
     output token, the two adjacent-x pixel pairs at rows y0 / y0+1 (token-major
     layout: token on partition, 2x256 channels on free dim).  DVE blends the 4
     corners with per-token weights (free-dim step-0 broadcast APs).  A second
     SBUF-source transpose-mode dma_gather flips the blended 128x(8x256) tile to
     channel-major [c, token].  TensorE accumulates W_k^T @ cols into PSUM over
     all 9 taps x 2 c-halves.
  3. PSUM -> SBUF -> HBM fp32 output.

The x-pair gather fetches columns (b, b+1) with b = clip(x0, 0, 62); the
left/right corner weights are remapped (wA/wB) so image-boundary tokens whose
x0 is -1 or 63 still read the right column, with out-of-image corners weighted
zero exactly as the reference's `valid` mask does.
"""

import os

import numpy as np

import concourse.bacc as bacc
import concourse.bass as bass
import concourse.mybir as mybir
from concourse.bass import AP, ts
from concourse.bass_utils import run_bass_kernel_spmd
from concourse.masks import make_identity
from concourse.tile import TileContext

FP32 = mybir.dt.float32
BF16 = mybir.dt.bfloat16
I16 = mybir.dt.int16
I32 = mybir.dt.int32

B = 8
C = 256
H = W = 64
HW = H * W           # 4096 pixels / tokens per tap
K = 9                # 3x3 taps
COUT = 256
NCH = 1024           # tokens per chunk
NCHUNKS = HW // NCH  # 4
A = mybir.AluOpType


def _bc(ap, shape):
    """Broadcast an AP to `shape` by appending/zero-striding trailing dims."""
    while ap.ndim < len(shape):
        ap = ap.unsqueeze(ap.ndim)
    return ap.broadcast_to(shape)


def build_nc() -> bass.Bass:
    nc = bacc.Bacc(target_bir_lowering=False)

    x_in = nc.dram_tensor("x", [C, HW], FP32, kind="ExternalInput")
    off_in = nc.dram_tensor("off", [2 * K, HW], FP32, kind="ExternalInput")
    # host pre-permutes weight to [cout, (k, c)] layout
    w_in = nc.dram_tensor("w", [COUT, K * C], FP32, kind="ExternalInput")
    out_d = nc.dram_tensor("out", [COUT, HW], FP32, kind="ExternalOutput")

    # channels-last bf16 image (+2 pad rows so an x-pair read at pixel 4095
    # stays in bounds)
    xcl_d = nc.dram_tensor("xcl", [HW + 2, C], BF16, kind="Internal")
    # token-major -> wrapped-16 index staging
    ilin_d = nc.dram_tensor("ilin", [128, 2 * K * HW // 128], I16, kind="Internal")

    with TileContext(nc) as tc:
        with tc.tile_pool(name="const", bufs=1) as cp:
            ident = cp.tile([128, 128], BF16)
            make_identity(nc, ident)
            identf = cp.tile([128, 128], FP32)
            make_identity(nc, identf)
            # lhsT tiles: block kc2 = k*2 + chalf holds rows c (128) x cout (256)
            wT = cp.tile([128, 2 * K, 256], BF16)
            # corner weights: (k, yrow, jcol, xpos) per token (p = partition)
            wpack = cp.tile([128, K, 2, HW // 128, 2, 2], BF16)
            # wrapped-16 gather indices per (k, yrow)
            idx_w = cp.tile([128, 2 * K, HW // 16], I16)
            # wrapped-16 iota 0..NCH-1 for the sbuf transpose-gather
            titer = cp.tile([128, NCH // 16], I16)

            with (
                tc.tile_pool(name="prep", bufs=1) as pp,
                tc.tile_pool(name="ppsum", bufs=4, space="PSUM") as pps,
            ):
                # ---- x -> channels-last bf16 in HBM ----
                x_bf = pp.tile([128, 2, HW], BF16)
                nc.gpsimd.dma_start(
                    out=x_bf[:], in_=x_in[:, :].rearrange("(h p) c -> p h c", p=128)
                )
                xcl_sb = pp.tile([128, HW // 128, C], BF16)
                for s in range(HW // 128):
                    pt = pps.tile([128, 2, 128], BF16, tag="pt")
                    for ch in range(2):
                        nc.tensor.transpose(
                            pt[:, ch, :], x_bf[:, ch, ts(s, 128)], ident[:]
                        )
                    nc.vector.tensor_copy(xcl_sb[:, s, :], pt[:])
                    if s % 8 == 7:
                        q4 = s // 8
                        nc.sync.dma_start(
                            out=xcl_d[q4 * 1024 : (q4 + 1) * 1024, :].rearrange(
                                "(s q) c -> q s c", q=128
                            ),
                            in_=xcl_sb[:, q4 * 8 : (q4 + 1) * 8, :],
                        )
                zpad = pp.tile([128, C], BF16)
                nc.vector.memset(zpad[:], 0.0)
                nc.sync.dma_start(out=xcl_d[HW : HW + 2, :], in_=zpad[0:2, :])

                # ---- offsets -> indices + corner weights ----
                # natural load (18 contiguous 16KB descriptors), then PE
                # transpose to token-major [128, j, r] to avoid the
                # 74K-single-element-descriptor DMA pattern
                off_nat = pp.tile([128, HW], FP32)
                nc.sync.dma_start(out=off_nat[0 : 2 * K, :], in_=off_in[:, :])
                off_t2 = pp.tile([128, HW // 128, 2 * K], FP32)
                for s4 in range(HW // 128 // 4):
                    ptf = pps.tile([128, 4, 2 * K], FP32, tag="ptf")
                    for i in range(4):
                        nc.tensor.transpose(
                            ptf[:, i, :],
                            off_nat[0 : 2 * K, ts(4 * s4 + i, 128)],
                            identf[0 : 2 * K, 0 : 2 * K],
                        )
                    nc.vector.tensor_copy(off_t2[:, 4 * s4 : 4 * s4 + 4, :], ptf[:])
                off_r = off_t2[:, :, :].rearrange("p j (k s) -> p s k j", s=2)
                oy = off_r[:, 0]  # [128, 9, 32] (k, j) strides (2, 18)
                ox = off_r[:, 1]

                NJ = HW // 128  # 32
                shp = [128, K, NJ]

                def f32(tag):
                    return pp.tile(shp, FP32, tag=tag, name=tag)

                # iotas
                it_j = pp.tile([128, NJ], I32)
                nc.gpsimd.iota(it_j[:], [[1, NJ]], base=0, channel_multiplier=0)
                jf = pp.tile([128, NJ], FP32)
                nc.vector.tensor_copy(jf[:], it_j[:])
                it_p = pp.tile([128, 1], I32)
                nc.gpsimd.iota(it_p[:], [[0, 1]], base=0, channel_multiplier=1)
                pf = pp.tile([128, 1], FP32)
                nc.vector.tensor_copy(pf[:], it_p[:])
                it_ky = pp.tile([128, 3, 3, NJ], I32)
                nc.gpsimd.iota(
                    it_ky[:], [[1, 3], [0, 3], [0, NJ]], base=0, channel_multiplier=0
                )
                kyf = pp.tile(shp, FP32, tag="kyf")
                nc.vector.tensor_copy(
                    kyf[:, :, :].rearrange("p (a b) j -> p a b j", a=3), it_ky[:]
                )
                it_kx = pp.tile([128, 3, 3, NJ], I32)
                nc.gpsimd.iota(
                    it_kx[:], [[0, 3], [1, 3], [0, NJ]], base=0, channel_multiplier=0
                )
                kxf = pp.tile(shp, FP32, tag="kxf")
                nc.vector.tensor_copy(
                    kxf[:, :, :].rearrange("p (a b) j -> p a b j", a=3), it_kx[:]
                )

                # ho = 2*j + p//64 ; wo = p%64  (token t = j*128 + p)
                t1 = pp.tile([128, 1], FP32, tag="t1")
                nc.vector.tensor_scalar(t1[:], pf[:], 1.0 / 64.0, None, A.mult)
                t2 = pp.tile([128, 1], FP32, tag="t2")
                nc.vector.tensor_scalar(t2[:], t1[:], 8388608.0, 8388608.0, A.add, A.subtract)
                p64 = pp.tile([128, 1], FP32, tag="p64")
                nc.vector.tensor_tensor(p64[:], t2[:], t1[:], A.is_gt)
                nc.vector.tensor_tensor(p64[:], t2[:], p64[:], A.subtract)
                wo = pp.tile([128, 1], FP32, tag="wo")
                nc.vector.tensor_scalar(wo[:], p64[:], -64.0, None, A.mult)
                nc.vector.tensor_tensor(wo[:], wo[:], pf[:], A.add)
                ho = pp.tile([128, NJ], FP32, tag="ho")
                nc.vector.tensor_scalar(ho[:], jf[:], 2.0, p64[:, 0:1], A.mult, A.add)

                # biased sample coords: pyb = oy + ky + ho + 7  (bias +8, base -1)
                pyb = f32("pyb")
                nc.vector.tensor_tensor(pyb[:], oy, kyf[:], A.add)
                nc.vector.scalar_tensor_tensor(
                    pyb[:], pyb[:], 7.0, ho[:].unsqueeze(1).broadcast_to(shp), A.add, A.add
                )
                pxb = f32("pxb")
                nc.vector.tensor_tensor(pxb[:], ox, kxf[:], A.add)
                nc.vector.scalar_tensor_tensor(
                    pxb[:], pxb[:], 7.0, wo[:].unsqueeze(1).broadcast_to(shp), A.add, A.add
                )

                fy = f32("fy")
                y0b = f32("y0b")
                nc.vector.tensor_scalar(fy[:], pyb[:], 8388608.0, 8388608.0, A.add, A.subtract)
                nc.vector.tensor_tensor(y0b[:], fy[:], pyb[:], A.is_gt)
                nc.vector.tensor_tensor(y0b[:], fy[:], y0b[:], A.subtract)
                nc.vector.tensor_tensor(fy[:], pyb[:], y0b[:], A.subtract)
                fx = f32("fx")
                x0b = f32("x0b")
                nc.vector.tensor_scalar(fx[:], pxb[:], 8388608.0, 8388608.0, A.add, A.subtract)
                nc.vector.tensor_tensor(x0b[:], fx[:], pxb[:], A.is_gt)
                nc.vector.tensor_tensor(x0b[:], fx[:], x0b[:], A.subtract)
                nc.vector.tensor_tensor(fx[:], pxb[:], x0b[:], A.subtract)

                ta = f32("ta")
                tb = f32("tb")
                # gather indices: p_r = clip(y_r,0,63)*64 + clip(x0,0,62)
                xcc = f32("xcc")
                nc.vector.tensor_scalar(xcc[:], x0b[:], 8.0, 0.0, A.subtract, A.max)
                nc.vector.tensor_scalar(xcc[:], xcc[:], 62.0, None, A.min)
                idx16 = pp.tile([128, K, 2, NJ], I16)
                for r in range(2):
                    nc.vector.tensor_scalar(
                        ta[:], y0b[:], 8.0 - r, 0.0, A.subtract, A.max
                    )
                    nc.vector.tensor_scalar(ta[:], ta[:], 63.0, None, A.min)
                    nc.vector.scalar_tensor_tensor(
                        tb[:], ta[:], 64.0, xcc[:], A.mult, A.add
                    )
                    nc.vector.tensor_copy(idx16[:, :, r, :], tb[:])

                # token-major [p,(k,r,j)] -> p-major HBM staging (contiguous
                # 1.1KB per partition)
                nc.sync.dma_start(out=ilin_d[:, :], in_=idx16[:])
                # wrapped-16 load into group 0 (per tap): idx for (kr, c)
                # lives at p = (16c + p16) % 128, j = (16c + p16) // 128,
                # decomposed affine as c = 8*ch + cl
                nc.sync.dma_start(
                    out=idx_w[0:16, :, :].rearrange(
                        "p kr (ch cl) -> p kr ch cl", cl=8
                    ),
                    in_=AP(ilin_d, 0, [[576, 16], [32, 2 * K], [1, 32], [16 * 576, 8]]),
                )
                for g in range(1, 8):
                    nc.sync.dma_start(
                        out=idx_w[16 * g : 16 * (g + 1), :, :],
                        in_=idx_w[0:16, :, :],
                    )

                # wy0 = (1-fy)*[8<=y0b<=71], wy1 = fy*[7<=y0b<=70]
                nc.vector.tensor_scalar(ta[:], y0b[:], 8.0, None, A.is_ge)
                nc.vector.tensor_scalar(tb[:], y0b[:], 71.0, None, A.is_le)
                vy0 = f32("vy0")
                nc.vector.tensor_tensor(vy0[:], ta[:], tb[:], A.mult)
                nc.vector.tensor_scalar(ta[:], y0b[:], 7.0, None, A.is_ge)
                nc.vector.tensor_scalar(tb[:], y0b[:], 70.0, None, A.is_le)
                vy1 = f32("vy1")
                nc.vector.tensor_tensor(vy1[:], ta[:], tb[:], A.mult)
                u0 = f32("u0")
                nc.vector.tensor_scalar(u0[:], fy[:], -1.0, 1.0, A.mult, A.add)
                wy0 = f32("wy0")
                nc.vector.tensor_tensor(wy0[:], u0[:], vy0[:], A.mult)
                wy1 = f32("wy1")
                nc.vector.tensor_tensor(wy1[:], fy[:], vy1[:], A.mult)

                # x-pair weights: pair base b = clip(x0,0,62); position weights
                # wA = (1-fx)*[0<=x0<=62] + fx*[x0==-1]
                # wB = fx*[0<=x0<=62] + (1-fx)*[x0==63]
                nc.vector.tensor_scalar(ta[:], x0b[:], 8.0, None, A.is_ge)
                nc.vector.tensor_scalar(tb[:], x0b[:], 70.0, None, A.is_le)
                e0 = f32("e0")
                nc.vector.tensor_tensor(e0[:], ta[:], tb[:], A.mult)
                eL = f32("eL")
                nc.vector.tensor_scalar(eL[:], x0b[:], 7.0, None, A.is_equal)
                eR = f32("eR")
                nc.vector.tensor_scalar(eR[:], x0b[:], 71.0, None, A.is_equal)
                nc.vector.tensor_scalar(u0[:], fx[:], -1.0, 1.0, A.mult, A.add)
                wA = f32("wA")
                nc.vector.tensor_tensor(ta[:], u0[:], e0[:], A.mult)
                nc.vector.tensor_tensor(tb[:], fx[:], eL[:], A.mult)
                nc.vector.tensor_tensor(wA[:], ta[:], tb[:], A.add)
                wB = f32("wB")
                nc.vector.tensor_tensor(ta[:], fx[:], e0[:], A.mult)
                nc.vector.tensor_tensor(tb[:], u0[:], eR[:], A.mult)
                nc.vector.tensor_tensor(wB[:], ta[:], tb[:], A.add)

                # corner weights (bf16 cast on write); duplicated into
                # adjacent bf16 pairs so the blend's broadcast operand has a
                # stride-1 innermost dim (enables the DVE 2x_1p perf mode)
                for dup in range(2):
                    nc.vector.tensor_tensor(
                        wpack[:, :, 0, :, 0, dup], wy0[:], wA[:], A.mult
                    )
                    nc.vector.tensor_tensor(
                        wpack[:, :, 0, :, 1, dup], wy0[:], wB[:], A.mult
                    )
                    nc.vector.tensor_tensor(
                        wpack[:, :, 1, :, 0, dup], wy1[:], wA[:], A.mult
                    )
                    nc.vector.tensor_tensor(
                        wpack[:, :, 1, :, 1, dup], wy1[:], wB[:], A.mult
                    )

                # ---- weight lhsT (fp32 HWDGE load + DVE cast; after the x
                # chain so the gather-critical path isn't delayed) ----
                w_f32 = pp.tile([128, 2, K * C], FP32)
                nc.sync.dma_start(
                    out=w_f32[:], in_=w_in[:, :].rearrange("(h p) c -> p h c", p=128)
                )
                w_bf = pp.tile([128, 2, K * C], BF16)
                nc.vector.tensor_copy(w_bf[:], w_f32[:])
                for kc2 in range(2 * K):
                    ptw = pps.tile([128, 2, 128], BF16, tag="pt")
                    for oh in range(2):
                        nc.tensor.transpose(
                            ptw[:, oh, :], w_bf[:, oh, ts(kc2, 128)], ident[:]
                        )
                    nc.scalar.copy(wT[:, kc2, :], ptw[:])

                # static iota (wrapped-16) for the sbuf-source transpose gather
                it_t = pp.tile([128, NCH // 16], I32)
                nc.gpsimd.iota(
                    it_t[:], [[16, NCH // 16]], base=0, channel_multiplier=1
                )
                psh = pp.tile([128, 1], I32)
                nc.gpsimd.iota(psh[:], [[0, 1]], base=0, channel_multiplier=1)
                nc.vector.tensor_scalar(psh[:], psh[:], 4, None, A.arith_shift_right)
                nc.vector.tensor_scalar(psh[:], psh[:], 4, None, A.logical_shift_left)
                nc.vector.tensor_tensor(
                    it_t[:], it_t[:], psh[:, 0:1].broadcast_to([128, NCH // 16]),
                    A.subtract,
                )
                nc.vector.tensor_copy(titer[:], it_t[:])

            # ---------------- main loop ----------------
            xsrc = AP(xcl_d, 0, [[C, HW + 1], [1, 2 * C]])  # overlapping pair rows
            NJC = NCH // 128  # 8 j-columns per chunk
            nreg = nc.gpsimd.to_reg(NCH)

            with (
                tc.tile_pool(name="vp", bufs=4) as vp,
                tc.tile_pool(name="wp", bufs=3) as wpo,
                tc.tile_pool(name="cc", bufs=4) as ccp,
                tc.tile_pool(name="ob", bufs=2) as obp,
                tc.tile_pool(name="mps", bufs=2, space="PSUM") as mps,
            ):
                iters = [(nch, k) for nch in range(NCHUNKS) for k in range(K)]
                vts = {}

                def issue_gathers(i):
                    nch_, k_ = iters[i]
                    vt_ = []
                    for r in range(2):
                        v = vp.tile(
                            [128, NJC, 2 * C], BF16, tag=f"v{r}", name=f"v{r}"
                        )
                        if os.environ.get("KBISECT") == "1":
                            nc.vector.memset(v[:], 0.25)
                        else:
                            nc.gpsimd.dma_gather(
                                out_ap=v[:],
                                in_ap=xsrc,
                                idxs_ap=idx_w[
                                    :,
                                    2 * k_ + r,
                                    nch_ * (NCH // 16) : (nch_ + 1) * (NCH // 16),
                                ],
                                num_idxs=NCH,
                                num_idxs_reg=nreg,
                                elem_size=2 * C,
                                elem_step=C,
                            )
                        vt_.append(v)
                    vts[i] = vt_

                issue_gathers(0)
                issue_gathers(1)
                ps = None
                for it in range(len(iters)):
                    nch, k = iters[it]
                    if it + 2 < len(iters):
                        issue_gathers(it + 2)
                    if k == 0:
                        ps = [
                            [mps.tile([128, 512], FP32, tag=f"ps{oh}{n2}", name=f"ps{oh}{n2}") for n2 in range(2)]
                            for oh in range(2)
                        ]
                    vt = vts.pop(it)
                    if True:
                        wsl = []
                        for r in range(2):
                            w_ap = wpack[:, k, r, nch * NJC : (nch + 1) * NJC, :, :]
                            w_ap = w_ap.unsqueeze(3).broadcast_to(
                                [128, NJC, 2, C // 2, 2]
                            )
                            wsl.append(w_ap)
                        v5 = [
                            vt[r][:, :, :].rearrange(
                                "p j (q h two) -> p j q h two", q=2, two=2
                            )
                            for r in range(2)
                        ]
                        v4 = [
                            vt[r][:, :, :].rearrange("p j (q c) -> p j q c", q=2)
                            for r in range(2)
                        ]
                        ct = ccp.tile([128, NJC, C], BF16, tag="ct")
                        tt = ccp.tile([128, NJC, C], BF16, tag="tt")
                        if os.environ.get("KBISECT") == "3":
                            nc.vector.tensor_copy(ct[:], v4[0][:, :, 0])
                        else:
                            for r in range(2):
                                nc.vector.tensor_tensor(v5[r], v5[r], wsl[r], A.mult)
                            nc.vector.tensor_tensor(
                                ct[:], v4[0][:, :, 0], v4[0][:, :, 1], A.add
                            )
                            nc.vector.tensor_tensor(
                                tt[:], v4[1][:, :, 0], v4[1][:, :, 1], A.add
                            )
                            nc.vector.tensor_tensor(ct[:], ct[:], tt[:], A.add)

                        # transpose to channel-major via sbuf-source gather
                        cm = ccp.tile([128, 2, NCH], BF16, tag="cm")
                        if os.environ.get("KBISECT") in ("1", "2"):
                            nc.vector.memset(cm[:], 0.125)
                        else:
                            nc.gpsimd.dma_gather(
                                out_ap=cm[:],
                                in_ap=ct[:],
                                idxs_ap=titer[:],
                                num_idxs=NCH,
                                num_idxs_reg=nreg,
                                elem_size=C,
                                transpose=True,
                                single_packet=False,
                                queue_num=int(os.environ.get("KQN", "0")),
                                sbuf_tokens_per_rank=128,
                                sbuf_free_dim_per_rank=2 * C,
                            )

                        for oh in range(2):
                            for ch in range(2):
                                lhsT = wT[:, 2 * k + ch, ts(oh, 128)]
                                for n2 in range(2):
                                    nc.tensor.matmul(
                                        ps[oh][n2][:],
                                        lhsT,
                                        cm[:, ch, ts(n2, 512)],
                                        start=(k == 0 and ch == 0),
                                        stop=(k == K - 1 and ch == 1),
                                    )

                    if k != K - 1:
                        continue
                    ob = obp.tile([128, 2, NCH], FP32, tag="ob")
                    for oh in range(2):
                        for n2 in range(2):
                            nc.scalar.copy(ob[:, oh, ts(n2, 512)], ps[oh][n2][:])
                    nc.sync.dma_start(
                        out=out_d[:, nch * NCH : (nch + 1) * NCH].rearrange(
                            "(h q) n -> q h n", q=128
                        ),
                        in_=ob[:],
                    )
    nc.compile()
    return nc


_NC_CACHE = None


def _get_nc():
    global _NC_CACHE
    if _NC_CACHE is None:
        _NC_CACHE = build_nc()
    return _NC_CACHE


def kernel(x: np.ndarray, offset: np.ndarray, weight: np.ndarray) -> np.ndarray:
    return _run(x, offset, weight)[0]


def _run(x, offset, weight, **spmd_kwargs):
    assert x.shape == (B, C, H, W) and offset.shape == (B, 2 * K, H, W)
    nc = _get_nc()
    # [cout, cin, 3, 3] -> [cout, (k, c)]
    w_perm = np.ascontiguousarray(
        weight.reshape(COUT, C, K).transpose(0, 2, 1).reshape(COUT, K * C)
    ).astype(np.float32)
    in_maps = [
        {
            "x": np.ascontiguousarray(x[b].reshape(C, HW)).astype(np.float32),
            "off": np.ascontiguousarray(offset[b].reshape(2 * K, HW)).astype(
                np.float32
            ),
            "w": w_perm,
        }
        for b in range(B)
    ]
    res = run_bass_kernel_spmd(nc, in_maps, core_ids=list(range(B)), **spmd_kwargs)
    out = np.stack([res.results[b]["out"].reshape(COUT, H, W) for b in range(B)])
    return out.astype(np.float32), res


if __name__ == "__main__":
    d = np.load("/root/problem/inputs.npz")
    out = kernel(d["x"], d["offset"], d["weight"])
    ref = np.load("/root/problem/ref_out_np.npy")
    err = np.abs(out - ref).max()
    rel = err / np.abs(ref).max()
    print("absmax err:", err, "rel:", rel)
